# revision 31
# baseline (speedup 1.0000x reference)
"""CRF loss (dense Gaussian bilateral filter) on 8 Trainium2 NeuronCores.

Math: with feats f_i (coords/ALPHA ++ I/BETA), K[i,j] = exp(-0.5*||f_i-f_j||^2),
s = K @ 1, n = (s+EPS)^-1/2, H = softmax(U), v_c = n*H_c:
    loss = n^T K n - sum_c v_c^T K v_c
(uses sum_c H_c = 1; per-batch block-diagonal K).

Sharding: cores 0-3 -> batch 0, cores 4-7 -> batch 1. Within a batch, each
core owns a 1536-wide slice of the (6144-padded) output-row space and the
full contraction over all 5888-padded j. K tiles are computed once
(PE matmul for dot products in 3-way-split bf16, ScalarE exp) and kept in
SBUF as bf16 [j-part 128, i-free 1536] tiles. Row sums come FREE from the
EXP pass: activation accum_out gives per-j-partition partial sums over the
core's own i slice. The partials are AllGathered (fp16) across the 4-core
batch group in TWO chunks: chunk 0 (j-blocks 0-22) fires mid-pass-A and its
collective + n/W build hide under the EXP tail; chunk 1 fires right after
pass A, and pass B's first half (j-blocks 0-22) runs during its service
window. A tiny dummy collective at kernel start absorbs the collective
firmware's cold-start latency. The 5-channel weighted filter (n, n*H_c)
streams the stored K tiles through the PE once more (even jb: W stationary;
odd jb: E stationary through the LDWEIGHTS port, output transposed). The
scalar loss is reduced on host from per-core [5,1536] filter outputs + s.
"""

import numpy as np
import ml_dtypes

import concourse.bass as bass
import concourse.bacc as bacc
import concourse.tile as tile
import concourse.mybir as mybir
import concourse.bass_utils as bass_utils
from concourse.hw_specs import get_activation_tables

ALPHA = 5.0
BETA = 5.0
EPS = 1e-20

B = 2
C = 4
XD = YD = ZD = 18
N = XD * YD * ZD          # 5832
NJ = 5888                 # j padded to 46*128
NJB = 46                  # j blocks of 128
JC0 = 23                  # j-blocks in collective chunk 0
IW = 1458                 # i-rows per core (N/4 exactly)
NIC = 12                  # i chunks per core (last one 50 wide)
ICH = [(0, 512), (512, 1024), (1024, IW)]  # dot/evens col chunks
NEG = -120.0              # pad bias => exp -> 0

F32 = mybir.dt.float32
F16 = mybir.dt.float16
BF16 = mybir.dt.bfloat16

DUMMY_CC = True

TRACE = False
LAST_RESULT = None

_compiled = {}


def _build():
    nc = bacc.Bacc("TRN2", target_bir_lowering=False, debug=False, num_devices=8)

    fhat = nc.dram_tensor("fhat", [39, NJ], BF16, kind="ExternalInput")
    frhs = nc.dram_tensor("frhs", [39, IW], BF16, kind="ExternalInput")
    biasj = nc.dram_tensor("biasj", [128, NJB], F32, kind="ExternalInput")
    h1 = nc.dram_tensor("h1", [128, C * NJB], F32, kind="ExternalInput")
    out = nc.dram_tensor("out", [5, IW], F32, kind="ExternalOutput")
    yt = nc.dram_tensor("yt", [128, 5 * NIC], F32, kind="ExternalOutput")
    sout = nc.dram_tensor("sout", [128, NJB], F32, kind="ExternalOutput")

    chunks = [(0, JC0), (JC0, NJB)]  # [lo, hi) j-block ranges

    with tile.TileContext(nc) as tc:
        with (
            tc.tile_pool(name="const", bufs=1) as cp,
            tc.tile_pool(name="epool", bufs=1) as ep,
            tc.tile_pool(name="dram", bufs=1, space="DRAM") as dp,
        ):
            fhat_sb = cp.tile([39, NJ], BF16)
            frhs_sb = cp.tile([39, IW], BF16)
            bias_sb = cp.tile([128, NJB], F32)
            h1_sb = cp.tile([128, C * NJB], F32)
            eps_sb = cp.tile([128, 1], F32)
            y_sb = cp.tile([5, IW], F32)
            wu_src = cp.tile([128, 520], BF16)
            yt_sb = cp.tile([128, 5 * NIC], F32)
            e_sb = ep.tile([128, NJB * IW], BF16)

            spart = []   # per-chunk fp32 partial row sums
            sp16 = []    # fp16 staged partials
            gsb = []     # gathered 4-rank partials
            ssb = []     # summed s per chunk
            lnt = []     # ln temp / pair-sum temp
            nsbc = []    # n per chunk
            wsb = []     # W planes per chunk [128, 5*w] (plane-major)
            bounce = []
            gath = []
            for g, (lo, hi) in enumerate(chunks):
                w = hi - lo
                spart.append(cp.tile([128, w], F32, name=f"spart{g}"))
                sp16.append(cp.tile([128, w], F16, name=f"sp16_{g}"))
                gsb.append(cp.tile([128, 4 * w], F16, name=f"gsb{g}"))
                ssb.append(cp.tile([128, w], F32, name=f"ssb{g}"))
                lnt.append(cp.tile([128, w], F32, name=f"lnt{g}"))
                nsbc.append(cp.tile([128, w], F32, name=f"nsbc{g}"))
                wsb.append(cp.tile([128, 5 * w], BF16, name=f"wsb{g}"))
                bounce.append(dp.tile([128 * w], F16, name=f"bounce{g}"))
                gath.append(dp.tile([4 * 128 * w], F16, name=f"gath{g}"))

            # Split the input DMAs across the Sync and ACT HWDGE queues so
            # the first dot's operands land ~2us earlier.
            nc.sync.dma_start(frhs_sb[:, 0:256], frhs[:, 0:256])
            for c0, c1 in ICH[1:]:
                nc.sync.dma_start(frhs_sb[:, c0:c1], frhs[:, c0:c1])
            nc.scalar.dma_start(fhat_sb[:, 0:512], fhat[:, 0:512])
            nc.scalar.dma_start(frhs_sb[:, 256:512], frhs[:, 256:512])
            nc.scalar.dma_start(bias_sb[:], biasj[:])
            nc.sync.dma_start(fhat_sb[:, 512:2048], fhat[:, 512:2048])
            nc.sync.dma_start(fhat_sb[:, 2048:NJ], fhat[:, 2048:NJ])
            nc.sync.dma_start(h1_sb[:], h1[:])
            nc.vector.memset(eps_sb[:], EPS)
            nc.vector.memset(wu_src[:], 0.0)

            if DUMMY_CC:
                dmy_in = dp.tile([16], F32, name="dmy_in")
                dmy_out = dp.tile([64], F32, name="dmy_out")
                nc.gpsimd.collective_compute(
                    "AllGather",
                    mybir.AluOpType.bypass,
                    replica_groups=[[0, 1, 2, 3], [4, 5, 6, 7]],
                    ins=[dmy_in[:]],
                    outs=[dmy_out[:]],
                )

            # Preload the table set holding BOTH Exp and Ln so no ACT table
            # switches land on the critical path mid-kernel.
            _tabs = list(get_activation_tables("gen3"))
            _nlx = _tabs.index("natural_log_exp_and_others")
            nc.scalar.add_instruction(
                mybir.InstLoadActFuncSet(
                    name=f"I-{nc.next_id()}", act_func_set_id=_nlx
                )
            )

            def kick_collective(g):
                lo, hi = chunks[g]
                w = hi - lo
                nc.vector.tensor_copy(sp16[g][:, :], spart[g][:, :])
                nc.sync.dma_start(
                    bounce[g][:].rearrange("(p j) -> p j", j=w), sp16[g][:, :]
                )
                nc.gpsimd.collective_compute(
                    "AllGather",
                    mybir.AluOpType.bypass,
                    replica_groups=[[0, 1, 2, 3], [4, 5, 6, 7]],
                    ins=[bounce[g][:]],
                    outs=[gath[g][:]],
                )

            def finish_chunk(g):
                """Readback + 4-way sum + n + W planes for chunk g."""
                lo, hi = chunks[g]
                w = hi - lo
                nc.scalar.dma_start(
                    gsb[g][:, :].rearrange("p (r j) -> p r j", j=w),
                    gath[g][:].rearrange("(r p j) -> p r j", p=128, j=w),
                )
                # 4-way sum: pair-adds on DVE and Pool concurrently
                nc.vector.tensor_add(
                    ssb[g][:, :], gsb[g][:, 0:w], gsb[g][:, w : 2 * w]
                )
                nc.gpsimd.tensor_add(
                    lnt[g][:, :], gsb[g][:, 2 * w : 3 * w], gsb[g][:, 3 * w : 4 * w]
                )
                nc.vector.tensor_add(ssb[g][:, :], ssb[g][:, :], lnt[g][:, :])
                nc.scalar.activation(
                    lnt[g][:, :],
                    ssb[g][:, :],
                    mybir.ActivationFunctionType.Ln,
                    bias=eps_sb[:, 0:1],
                    scale=1.0,
                )
                # Exp writes plane 0 (n, bf16) directly; muls split DVE/Pool
                nc.scalar.activation(
                    wsb[g][:, 0:w],
                    lnt[g][:, :],
                    mybir.ActivationFunctionType.Exp,
                    scale=-0.5,
                )
                for c in range(C):
                    eng = nc.vector if c % 2 == 0 else nc.gpsimd
                    eng.tensor_mul(
                        wsb[g][:, w * (c + 1) : w * (c + 2)],
                        wsb[g][:, 0:w],
                        h1_sb[:, NJB * c + lo : NJB * c + hi],
                    )
                nc.sync.dma_start(sout[:, lo:hi], ssb[g][:, :])

            # ---- pass A: dot -> exp (accum_out = per-j partial row sums);
            # chunk 0's collective fires as soon as its partials are done ----
            with tc.tile_pool(name="dotp", bufs=2, space="PSUM") as dotp:
                for jb in range(NJB):
                    g = 0 if jb < JC0 else 1
                    lo = chunks[g][0]
                    dps = dotp.tile([128, IW], F32, tag="dot")
                    lw = fhat_sb[:, 128 * jb : 128 * (jb + 1)]
                    ich = [(0, 256), (256, 512)] + ICH[1:] if jb == 0 else ICH
                    for c0, c1 in ich:
                        nc.tensor.matmul(
                            dps[:, c0:c1],
                            lw,
                            frhs_sb[:, c0:c1],
                            start=True,
                            stop=True,
                        )
                    nc.scalar.activation(
                        e_sb[:, IW * jb : IW * (jb + 1)],
                        dps[:, :],
                        mybir.ActivationFunctionType.Exp,
                        bias=bias_sb[:, jb : jb + 1],
                        scale=1.0,
                        accum_out=spart[g][:, jb - lo : jb - lo + 1],
                    )
                    if jb == JC0 - 1:
                        kick_collective(0)

            kick_collective(1)
            finish_chunk(0)

            # ---- pass B: Y[5, IW] = W^T E accumulated over all j blocks.
            # Part 1 (chunk-0 j-blocks) runs during chunk 1's collective.
            # Even jb stream E through the PE rhs port (W stationary); odd jb
            # load E chunks through the LDWEIGHTS port and stream the tiny W
            # (output transposed, [i-chunk, 5] per chunk). Host adds the
            # transposed half back in.
            evens = [jb for jb in range(NJB) if jb % 3 == 0]
            odds = [jb for jb in range(NJB) if jb % 3 != 0]
            with tc.tile_pool(name="ypool", bufs=1, space="PSUM") as yp:
                y_ps = yp.tile([5, IW], F32)
                yt_ps = yp.tile([128, 5 * NIC], F32)
                wu_ps = yp.tile([5, 512], F32)
                nc.vector.memset(yt_ps[:, :], 0.0)
                # PE sits idle ~4.4us between the last dot and W0 — past the
                # HAM re-throttle window — so pass B would start at half
                # clock. Fill the gap with throwaway streams to keep K=8/8.
                for i in range(18):
                    nc.tensor.matmul(
                        wu_ps[:, :],
                        wu_src[:, 0:5],
                        wu_src[:, 8:520],
                        start=True,
                        stop=True,
                    )
                ne_seen = no_seen = 0
                for g, (lo, hi) in enumerate(chunks):
                    w = hi - lo
                    w_view = wsb[g][:, :].rearrange("p (r j) -> p r j", j=w)
                    for jb in [x for x in range(lo, hi) if x in evens] + [
                        x for x in range(lo, hi) if x in odds
                    ]:
                        if jb in evens:
                            ne_seen += 1
                            lwv = w_view[:, :, jb - lo]
                            for c0, c1 in ICH:
                                nc.tensor.matmul(
                                    y_ps[:, c0:c1],
                                    lwv,
                                    e_sb[:, IW * jb + c0 : IW * jb + c1],
                                    start=(ne_seen == 1),
                                    stop=(ne_seen == len(evens)),
                                )
                            if ne_seen == len(evens):
                                # Y complete; flush + DMA while odds still run
                                nc.scalar.copy(y_sb[:, :], y_ps[:, :])
                                nc.sync.dma_start(out[0:5, :], y_sb[:, :])
                        else:
                            no_seen += 1
                            for m in range(NIC):
                                m1 = min(128 * (m + 1), IW)
                                nc.tensor.matmul(
                                    yt_ps[0 : m1 - 128 * m, 5 * m : 5 * (m + 1)],
                                    e_sb[:, IW * jb + 128 * m : IW * jb + m1],
                                    w_view[:, :, jb - lo],
                                    start=False,
                                    stop=(no_seen == len(odds)),
                                    skip_group_check=True,
                                )
                    if g == 0:
                        finish_chunk(1)
                nc.vector.tensor_copy(yt_sb[:, :], yt_ps[:, :])
                nc.sync.dma_start(yt[:, :], yt_sb[:, :])

    nc.compile()
    return nc


def _split3(a):
    """3-way bf16 split: a ~ h + m + l to ~24 mantissa bits."""
    bf = ml_dtypes.bfloat16
    h = a.astype(bf)
    r1 = a - h.astype(np.float32)
    m = r1.astype(bf)
    l = (r1 - m.astype(np.float32)).astype(bf)
    return h, m, l


def kernel(I, U):
    global LAST_RESULT
    if "nc" not in _compiled:
        _compiled["nc"] = _build()
    nc = _compiled["nc"]

    I = np.asarray(I, np.float32)
    U = np.asarray(U, np.float32)

    g = np.arange(XD, dtype=np.float32)
    gx, gy, gz = np.meshgrid(g, g, g, indexing="ij")
    coords = np.stack([gx, gy, gz], 0).reshape(3, N)

    in_maps = []
    host = []
    for k in range(8):
        b, r = divmod(k, 4)
        feats = np.concatenate(
            [coords / ALPHA, I[b].reshape(3, N) / BETA], 0
        ).astype(np.float32)  # [6, N]
        sq = (feats.astype(np.float64) ** 2).sum(0)  # [N] f64
        shalf = (-0.5 * sq).astype(np.float32)
        bf = ml_dtypes.bfloat16
        fh, fm, fl = _split3(feats)
        s1, s2, s3 = _split3(shalf)

        one = np.ones((1, N), bf)
        fhat = np.zeros((39, NJ), bf)
        fhat[:, :N] = np.concatenate([fh, fh, fm, fh, fl, fm, one, one, one], 0)

        gi = IW * r + np.arange(IW)
        giv = gi
        frhs = np.concatenate(
            [
                fh[:, giv], fm[:, giv], fh[:, giv], fl[:, giv], fh[:, giv],
                fm[:, giv], s1[None, giv], s2[None, giv], s3[None, giv],
            ],
            0,
        ).astype(bf)

        bpad = np.full(NJ, NEG, np.float32)
        bpad[:N] = shalf
        biasj = bpad.reshape(NJB, 128).T.copy()  # [128, NJB]

        uf = U[b].reshape(C, N).astype(np.float64)
        uf = uf - uf.max(0, keepdims=True)
        e = np.exp(uf)
        H1 = (e / e.sum(0, keepdims=True)).astype(np.float32)  # [C, N]
        hpad = np.zeros((C, NJ), np.float32)
        hpad[:, :N] = H1
        h1in = np.concatenate(
            [hpad[c].reshape(NJB, 128).T for c in range(C)], axis=1
        ).copy()  # [128, C*NJB]

        in_maps.append(
            {"fhat": fhat, "frhs": frhs, "biasj": biasj, "h1": h1in}
        )
        host.append((H1, giv))

    res = bass_utils.run_bass_kernel_spmd(
        nc, in_maps, core_ids=list(range(8)), trace=TRACE
    )
    LAST_RESULT = res

    loss = 0.0
    for k in range(8):
        b, r = divmod(k, 4)
        H1, giv = host[k]
        o = res.results[k]["out"].astype(np.float64)
        ytk = res.results[k]["yt"].astype(np.float64)  # [128, 5*NIC]
        sfull = res.results[k]["sout"].astype(np.float64)  # [128, NJB]
        # yt[p, 5m+r] = Y_odd[r, 128m+p]; cols >= IW in the last 128-chunk
        # were never written (memset 0) and are dropped.
        yodd = ytk.reshape(128, NIC, 5).transpose(2, 1, 0).reshape(5, NIC * 128)
        yv = o[0:5] + yodd[:, :IW]  # [5, IW]
        # s for own i rows: global voxel i -> (partition i%128, block i//128)
        s = sfull[giv % 128, giv // 128]
        n = 1.0 / np.sqrt(s + EPS)
        hv = H1[:, giv].astype(np.float64)  # [C, nvalid]
        loss += (n * yv[0]).sum()
        for c in range(C):
            loss -= (n * hv[c] * yv[1 + c]).sum()
    return np.float32(loss)
